# revision 1
# baseline (speedup 1.0000x reference)
"""Trainium2 Bass kernel for a pre-LN transformer block (B=2, T=2048, C=512,
H=16 heads, HS=32, DF=2048), distributed over 8 NeuronCores.

Sharding strategy:
  Phase 0 (token-parallel): core c layernorms its 512-token block and
    PE-transposes it; AllGather produces hT = LN1(x)^T [C, 4096] on every core.
  Phase 1 (head-parallel): core c computes q^T,k^T [64, 4096] and v [4096, 64]
    for its 2 heads, then causal attention in transposed-score space:
    scores^T[s,t] (K=32 matmul), exp on ACT, causal mask via affine_select,
    and o_un^T/Z via a single matmul with a ones-column ([v|1] stationary).
  AllToAll: switches from head-sharded [o_un^T;Z] to token-sharded.
  Phase 2 (token-parallel): core c folds 1/Z into o^T columns, computes
    o @ Wo + x, LN2, FFN (relu(h2@W1)@W2) for its 512-token block.

All heavy matmuls run in float32r (tf32-like) at full PE rate.
"""
import numpy as np

import bass_rust
import concourse.bass as bass
import concourse.mybir as mybir
import concourse.tile as tile
from concourse.bass_utils import run_bass_kernel_spmd

B, T, C, H, HS = 2, 2048, 512, 16, 32
DF = 4 * C
EPS = 1e-3
NCORES = 8
NT = B * T          # 4096 flattened tokens
TB = NT // NCORES   # 512 tokens per core
P = 128
HPC = H // NCORES   # 2 heads per core
D2 = HPC * HS       # 64 = packed head dim per core
F32 = mybir.dt.float32
F32R = mybir.dt.float32r
BF16 = mybir.dt.bfloat16
AF = mybir.ActivationFunctionType
ALU = mybir.AluOpType

_ev_counter = [0]


def _split_excess_waits(nc, max_waits=1):
    """This walrus build rejects >1 semaphore wait per real instruction; Tile's
    kernel-tail drain (and occasionally other aggregation points) can exceed
    that. Hoist extra waits onto EventSemaphore instructions inserted
    immediately before, on the same engine."""
    n_split = 0
    for bb in nc.main_func.blocks:
        il = bb.instructions
        i = 0
        while i < len(il):
            inst = il[i]
            si = inst.sync_info
            if si is None:
                i += 1
                continue
            waits = list(si.on_wait)
            if len(waits) <= max_waits:
                i += 1
                continue
            keep, extra = waits[:max_waits], waits[max_waits:]
            evs = []
            for w in extra:
                _ev_counter[0] += 1
                ev = mybir.InstEventSemaphore(
                    name=f"EV-WSPLIT-{_ev_counter[0]}",
                    engine=inst.engine,
                    sync_info=bass_rust.SyncInfo(on_wait=[w], on_update=[]),
                )
                nc.register_instruction(ev)
                evs.append(ev)
            inst.sync_info = bass_rust.SyncInfo(
                on_wait=keep, on_update=list(si.on_update)
            )
            for k, ev in enumerate(evs):
                il.insert(i + k, ev)
            i += len(evs) + 1
            n_split += 1
    return n_split


def _build_nc(repeat=1, skip=()):
    nc = bass.Bass(num_devices=NCORES)

    # ---- per-core external inputs ----
    xblk = nc.declare_dram_parameter("xblk", [TB, C], F32, isOutput=False)
    wq = nc.declare_dram_parameter("wq", [C, D2], F32, isOutput=False)
    wk = nc.declare_dram_parameter("wk", [C, D2], F32, isOutput=False)
    wv = nc.declare_dram_parameter("wv", [C, D2], F32, isOutput=False)
    bq = nc.declare_dram_parameter("bq", [D2], F32, isOutput=False)
    bk = nc.declare_dram_parameter("bk", [D2], F32, isOutput=False)
    bv = nc.declare_dram_parameter("bv", [D2], F32, isOutput=False)
    wo = nc.declare_dram_parameter("wo", [C, C], F32, isOutput=False)
    bo = nc.declare_dram_parameter("bo", [C], F32, isOutput=False)
    w1 = nc.declare_dram_parameter("w1", [C, DF], F32, isOutput=False)
    b1r = nc.declare_dram_parameter("b1r", [DF // P, P], F32, isOutput=False)
    w2 = nc.declare_dram_parameter("w2", [DF, C], F32, isOutput=False)
    b2 = nc.declare_dram_parameter("b2", [C], F32, isOutput=False)
    g1 = nc.declare_dram_parameter("g1", [C], F32, isOutput=False)
    be1 = nc.declare_dram_parameter("be1", [C], F32, isOutput=False)
    g2 = nc.declare_dram_parameter("g2", [C], F32, isOutput=False)
    be2 = nc.declare_dram_parameter("be2", [C], F32, isOutput=False)
    out = nc.declare_dram_parameter("out", [TB, C], F32, isOutput=True)

    ident_dram = nc.inline_tensor(np.eye(P, dtype=np.float32), name="ident_c")

    NCT = C // P   # 4 c-tiles
    NFT = DF // P  # 16 f-tiles
    NTT = TB // P  # 4 token tiles per block
    VS = 2 * (HS + 1)  # 66: per s-tile v layout [v_h0(32) | 1 | v_h1(32) | 1]
    NST = T // P   # 16 s-tiles per batch

    with tile.TileContext(nc) as tc:
        import contextlib

        with contextlib.ExitStack() as ctx:
            const = ctx.enter_context(tc.tile_pool(name="const", bufs=1))
            persist = ctx.enter_context(tc.tile_pool(name="persist", bufs=1))
            dram = ctx.enter_context(tc.tile_pool(name="dram", bufs=1, space="DRAM"))

            # ---- constants / broadcasts ----
            ident = const.tile([P, P], F32, name="ident")
            nc.sync.dma_start(out=ident, in_=ident_dram[:, :])
            eps_t = const.tile([P, 1], F32, name="eps_t")
            nc.vector.memset(eps_t, EPS)

            def bcast(name, src):
                t = const.tile([P, C], F32, name=name)
                nc.sync.dma_start(out=t, in_=src[:].partition_broadcast(P))
                return t

            g1b = bcast("g1b", g1)
            be1b = bcast("be1b", be1)
            g2b = bcast("g2b", g2)
            be2b = bcast("be2b", be2)
            bob = bcast("bob", bo)
            b2b = bcast("b2b", b2)
            bvb = const.tile([P, D2], F32, name="bvb")
            nc.sync.dma_start(out=bvb, in_=bv[:].partition_broadcast(P))
            bq_sb = const.tile([D2, 1], F32, name="bq_sb")
            nc.sync.dma_start(out=bq_sb, in_=bq[:].unsqueeze(1))
            bk_sb = const.tile([D2, 1], F32, name="bk_sb")
            nc.sync.dma_start(out=bk_sb, in_=bk[:].unsqueeze(1))
            b1cols = const.tile([P, NFT], F32, name="b1cols")
            for f in range(NFT):
                nc.sync.dma_start(out=b1cols[:, f : f + 1], in_=b1r[f, :].unsqueeze(1))

            # ---- weights resident in SBUF (fp32r) ----
            wq_sb = const.tile([P, NCT, D2], BF16, name="wq_sb")
            wk_sb = const.tile([P, NCT, D2], BF16, name="wk_sb")
            wv_sb = const.tile([P, NCT, D2], BF16, name="wv_sb")
            wqkv_st = const.tile([P, 3 * D2], F32, name="wqkv_st")
            for j in range(NCT):
                nc.sync.dma_start(out=wqkv_st[:, 0:D2], in_=wq[j * P : (j + 1) * P, :])
                nc.sync.dma_start(
                    out=wqkv_st[:, D2 : 2 * D2], in_=wk[j * P : (j + 1) * P, :]
                )
                nc.sync.dma_start(
                    out=wqkv_st[:, 2 * D2 : 3 * D2], in_=wv[j * P : (j + 1) * P, :]
                )
                nc.vector.tensor_copy(wq_sb[:, j, :], wqkv_st[:, 0:D2])
                nc.vector.tensor_copy(wk_sb[:, j, :], wqkv_st[:, D2 : 2 * D2])
                nc.vector.tensor_copy(wv_sb[:, j, :], wqkv_st[:, 2 * D2 : 3 * D2])
            wo_sb = const.tile([P, NCT, C], BF16, name="wo_sb")
            wo_st = const.tile([P, C], F32, name="wo_st")
            for j in range(NCT):
                nc.sync.dma_start(out=wo_st, in_=wo[j * P : (j + 1) * P, :])
                nc.vector.tensor_copy(wo_sb[:, j, :], wo_st)
            w1_sb = const.tile([P, NCT, DF], F32R, name="w1_sb")
            for j in range(NCT):
                nc.sync.dma_start(
                    out=w1_sb[:, j, :], in_=w1[j * P : (j + 1) * P, :].bitcast(F32R)
                )
            w2_sb = const.tile([P, NFT, C], F32R, name="w2_sb")
            for f in range(NFT):
                nc.sync.dma_start(
                    out=w2_sb[:, f, :], in_=w2[f * P : (f + 1) * P, :].bitcast(F32R)
                )
            for _rep in range(repeat):
                # ---- DRAM comm buffers (per repeat: Shared tensors need a single writer) ----
                hT_cc_in = dram.tile([C, TB], BF16, name="hT_cc_in")
                hT_all = dram.tile([NCORES, C, TB], BF16, name="hT_all", addr_space="Shared")
                a2a_in = dram.tile([NCORES, VS, TB], BF16, name="a2a_in")
                a2a_out = dram.tile([NCORES, VS, TB], BF16, name="a2a_out")
                rz_dram = dram.tile([H, TB], F32, name="rz_dram")
                # ================= Phase 0: LN1 on my token block + transpose ====
                x_sb = persist.tile([P, NTT, C], F32, name="x_sb")  # kept for residual
                with (
                    tc.tile_pool(name="ph0", bufs=2) as ph0,
                    tc.tile_pool(name="ph0ps", bufs=2, space="PSUM") as ph0ps,
                ):
                    hT_sb = ph0.tile([P, NCT, TB], BF16, name="hT_sb")
                    for i in range(NTT):
                        nc.sync.dma_start(
                            out=x_sb[:, i, :], in_=xblk[i * P : (i + 1) * P, :]
                        )
                        stats = ph0.tile([P, 6], F32, name="stats0")
                        nc.vector.bn_stats(out=stats, in_=x_sb[:, i, :])
                        mv = ph0.tile([P, 2], F32, name="mv0")
                        nc.vector.bn_aggr(out=mv, in_=stats)
                        rstd = ph0.tile([P, 1], F32, name="rstd0")
                        nc.scalar.activation(
                            out=rstd, in_=mv[:, 1:2], func=AF.Sqrt, bias=eps_t
                        )
                        nc.vector.reciprocal(out=rstd, in_=rstd)
                        h_t = ph0.tile([P, C], F32, name="h_t0")
                        nc.vector.tensor_scalar(
                            out=h_t,
                            in0=x_sb[:, i, :],
                            scalar1=mv[:, 0:1],
                            scalar2=rstd,
                            op0=ALU.subtract,
                            op1=ALU.mult,
                        )
                        nc.vector.tensor_mul(h_t, h_t, g1b)
                        nc.vector.tensor_add(h_t, h_t, be1b)
                        for j in range(NCT):
                            tr_ps = ph0ps.tile([P, P], F32, name="tr_ps0")
                            nc.tensor.transpose(
                                tr_ps[:], h_t[:, j * P : (j + 1) * P], ident[:]
                            )
                            nc.vector.tensor_copy(
                                hT_sb[:, j, i * P : (i + 1) * P], tr_ps[:]
                            )
                    for j in range(NCT):
                        nc.sync.dma_start(
                            out=hT_cc_in[j * P : (j + 1) * P, :],
                            in_=hT_sb[:, j, :],
                        )
                if "ag" in skip:
                    nc.sync.dma_start(out=hT_all[0, :, :], in_=hT_cc_in[:, :])
                else:
                    nc.gpsimd.collective_compute(
                        "AllGather",
                        ALU.bypass,
                        replica_groups=[list(range(NCORES))],
                        ins=[hT_cc_in[:, :]],
                        outs=[hT_all[:, :, :]],
                    )

                # ================= Phase 1: QKV for my 2 heads over all tokens ===
                p1ctx = contextlib.ExitStack()
                p1big = p1ctx.enter_context(tc.tile_pool(name="p1big", bufs=1))
                qT_sb = p1big.tile([D2, NT], BF16, name="qT_sb")
                kT_sb = p1big.tile([D2, NT], BF16, name="kT_sb")
                v_sb = p1big.tile([P, 2 * NST, VS], BF16, name="v_sb")
                # rows 64*hh .. 64*hh+31: o_un for head hh; row 64*hh+32: Z
                oz_sb = p1big.tile([P, NT], BF16, name="oz_sb")
                # ones columns for the [v | 1] stationary operand
                ones_f = p1big.tile([P, 1], F32, name="ones_f")
                nc.vector.memset(ones_f, 1.0)
                ones_r = p1big.tile([P, 1], BF16, name="ones_r")
                nc.vector.tensor_copy(ones_r, ones_f)
                for g in range(2 * NST):
                    nc.vector.tensor_copy(v_sb[:, g, HS : HS + 1], ones_r)
                    nc.vector.tensor_copy(v_sb[:, g, VS - 1 : VS], ones_r)

                with (
                    tc.tile_pool(name="ph1", bufs=4) as ph1,
                    tc.tile_pool(name="ph1ps", bufs=2, space="PSUM") as ph1ps,
                ):
                    for n in range(NT // TB):  # 8 chunks of 512 tokens
                        rhs = ph1.tile([P, NCT, TB], BF16, name="rhs1")
                        for j in range(NCT):
                            nc.sync.dma_start(
                                out=rhs[:, j, :],
                                in_=hT_all[n, j * P : (j + 1) * P, :],
                            )
                        ps_q = ph1ps.tile([D2, TB], F32, name="ps_q")
                        for j in range(NCT):
                            nc.tensor.matmul(
                                ps_q[:],
                                wq_sb[:, j, :],
                                rhs[:, j, :],
                                start=(j == 0),
                                stop=(j == NCT - 1),
                            )
                        nc.vector.tensor_scalar(
                            out=qT_sb[:, n * TB : (n + 1) * TB],
                            in0=ps_q[:],
                            scalar1=bq_sb,
                            scalar2=None,
                            op0=ALU.add,
                        )
                        ps_k = ph1ps.tile([D2, TB], F32, name="ps_k")
                        for j in range(NCT):
                            nc.tensor.matmul(
                                ps_k[:],
                                wk_sb[:, j, :],
                                rhs[:, j, :],
                                start=(j == 0),
                                stop=(j == NCT - 1),
                            )
                        nc.vector.tensor_scalar(
                            out=kT_sb[:, n * TB : (n + 1) * TB],
                            in0=ps_k[:],
                            scalar1=bk_sb,
                            scalar2=None,
                            op0=ALU.add,
                        )
                        for m in range(NTT):
                            g = n * NTT + m  # global token-tile index
                            ps_v = ph1ps.tile([P, D2], F32, name="ps_v")
                            for j in range(NCT):
                                nc.tensor.matmul(
                                    ps_v[:],
                                    rhs[:, j, m * P : (m + 1) * P],
                                    wv_sb[:, j, :],
                                    start=(j == 0),
                                    stop=(j == NCT - 1),
                                )
                            vv = ph1.tile([P, D2], F32, name="vv1")
                            nc.vector.tensor_add(vv, ps_v[:], bvb)
                            nc.vector.tensor_copy(v_sb[:, g, 0:HS], vv[:, 0:HS])
                            nc.vector.tensor_copy(
                                v_sb[:, g, HS + 1 : 2 * HS + 1], vv[:, HS:D2]
                            )

                # ================= Phase 1b: attention in transposed-score space =
                if "attn" in skip:
                    nc.gpsimd.memset(oz_sb[:, 0:2048], 1.0)
                    nc.gpsimd.memset(oz_sb[:, 2048:4096], 1.0)
                with (
                    tc.tile_pool(name="att", bufs=5) as att,
                    tc.tile_pool(name="attps", bufs=3, space="PSUM") as attps,
                    tc.tile_pool(name="attpso", bufs=2, space="PSUM") as attpso,
                ):
                    for b in range(B if "attn" not in skip else 0):
                        for hh in range(HPC):
                            qrows = slice(hh * HS, (hh + 1) * HS)
                            for ci in range(T // TB):  # 4 t-chunks of 512
                                t0 = b * T + ci * TB
                                ns = NTT * ci + NTT  # s-tiles for this chunk
                                ps_o = attpso.tile([HS + 1, TB], F32, name="ps_o")
                                for jp in range(0, ns, 2):
                                    ps_s = attps.tile([P, 2 * TB], F32, name="ps_s")
                                    for u in range(2):
                                        j = jp + u
                                        s0 = b * T + j * P
                                        nc.tensor.matmul(
                                            ps_s[:, u * TB : (u + 1) * TB],
                                            kT_sb[qrows, s0 : s0 + P],
                                            qT_sb[qrows, t0 : t0 + TB],
                                            start=True,
                                            stop=True,
                                        )
                                    e_t = att.tile([P, 2 * TB], BF16, name="e_t")
                                    nc.scalar.activation(out=e_t, in_=ps_s[:], func=AF.Exp)
                                    for u in range(2):
                                        j = jp + u
                                        off = j - NTT * ci
                                        if off >= 0:
                                            # only the [128,128] diagonal block
                                            # needs the mask (f - p >= 0 there);
                                            # cols < 128*off are excluded from
                                            # o_un below
                                            nc.gpsimd.affine_select(
                                                out=e_t[:, u * TB + off * P : u * TB + (off + 1) * P],
                                                in_=e_t[:, u * TB + off * P : u * TB + (off + 1) * P],
                                                compare_op=ALU.is_ge,
                                                fill=0.0,
                                                base=0,
                                                pattern=[[1, P]],
                                                channel_multiplier=-1,
                                            )
                                    for u in range(2):
                                        j = jp + u
                                        off = max(j - NTT * ci, 0)
                                        g = b * NST + j
                                        vb = hh * (HS + 1)
                                        nc.tensor.matmul(
                                            ps_o[:, off * P : TB],
                                            v_sb[:, g, vb : vb + HS + 1],
                                            e_t[:, u * TB + off * P : (u + 1) * TB],
                                            start=(jp == 0 and u == 0),
                                            stop=(jp + u == ns - 1),
                                        )
                                nc.vector.tensor_copy(
                                    oz_sb[64 * hh : 64 * hh + HS + 1, t0 : t0 + TB],
                                    ps_o[:, :],
                                )

                # ================= AllToAll: head-sharded -> token-sharded =======
                # pack [o_h0;Z_h0] rows 0..32 and [o_h1;Z_h1] rows 64..96 into 66 rows
                for k in range(NCORES):
                    for hh in range(HPC):
                        nc.sync.dma_start(
                            out=a2a_in[k, 33 * hh : 33 * hh + 33, :],
                            in_=oz_sb[64 * hh : 64 * hh + 33, k * TB : (k + 1) * TB],
                        )
                p1ctx.close()
                if "a2a" in skip:
                    nc.sync.dma_start(out=a2a_out[:, :, :], in_=a2a_in[:, :, :])
                else:
                    nc.gpsimd.collective_compute(
                        "AllToAll",
                        ALU.bypass,
                        replica_groups=[list(range(NCORES))],
                        ins=[a2a_in[:, :, :]],
                        outs=[a2a_out[:, :, :]],
                    )

                # ================= Phase 2: Wo + residual + LN2 + FFN ===========
                with (
                    tc.tile_pool(name="ph2", bufs=2) as ph2,
                    tc.tile_pool(name="ph2w", bufs=1) as ph2w,
                ):
                    oT = ph2w.tile([P, NCT, TB], BF16, name="oT")
                    z_sb = ph2w.tile([H, TB], BF16, name="z_sb")
                    for j in range(NCORES):
                        for hh in range(HPC):
                            pb = 64 * (j % 2) + 32 * hh
                            nc.sync.dma_start(
                                out=oT[pb : pb + HS, j // 2, :],
                                in_=a2a_out[j, 33 * hh : 33 * hh + HS, :],
                            )
                            nc.sync.dma_start(
                                out=z_sb[2 * j + hh : 2 * j + hh + 1, :],
                                in_=a2a_out[j, 33 * hh + HS : 33 * hh + HS + 1, :],
                            )
                    rz_sb = ph2w.tile([H, TB], F32, name="rz_sb")
                    nc.vector.reciprocal(out=rz_sb, in_=z_sb)
                    nc.sync.dma_start(out=rz_dram[:, :], in_=rz_sb[:, :])
                    rmat = ph2w.tile([P, NCT, TB], F32, name="rmat")
                    for hh2 in range(H):
                        nc.sync.dma_start(
                            out=rmat[32 * (hh2 % 4) : 32 * (hh2 % 4) + 32, hh2 // 4, :],
                            in_=rz_dram[hh2, :].partition_broadcast(32),
                        )
                    rmatb = ph2w.tile([P, NCT, TB], BF16, name="rmatb")
                    for j in range(NCT):
                        nc.vector.tensor_copy(rmatb[:, j, :], rmat[:, j, :])
                        nc.vector.tensor_mul(oT[:, j, :], oT[:, j, :], rmatb[:, j, :])

                    # attn out + residual + LN2 -> h2T
                    x2_sb = ph2w.tile([P, NTT, C], F32, name="x2_sb")
                    h2T = ph2w.tile([P, NCT, TB], F32R, name="h2T")
                    with tc.tile_pool(name="ph2psA", bufs=2, space="PSUM") as ph2psA:
                        for m in range(NTT):
                            ps_a = ph2psA.tile([P, C], F32, name="ps_a")
                            for j in range(NCT):
                                nc.tensor.matmul(
                                    ps_a[:],
                                    oT[:, j, m * P : (m + 1) * P],
                                    wo_sb[:, j, :],
                                    start=(j == 0),
                                    stop=(j == NCT - 1),
                                )
                            nc.vector.tensor_add(x2_sb[:, m, :], ps_a[:], x_sb[:, m, :])
                            nc.vector.tensor_add(x2_sb[:, m, :], x2_sb[:, m, :], bob)
                            stats2 = ph2.tile([P, 6], F32, name="stats2")
                            nc.vector.bn_stats(out=stats2, in_=x2_sb[:, m, :])
                            mv2 = ph2.tile([P, 2], F32, name="mv2")
                            nc.vector.bn_aggr(out=mv2, in_=stats2)
                            rstd2 = ph2.tile([P, 1], F32, name="rstd2")
                            nc.scalar.activation(
                                out=rstd2, in_=mv2[:, 1:2], func=AF.Sqrt, bias=eps_t
                            )
                            nc.vector.reciprocal(out=rstd2, in_=rstd2)
                            h2_t = ph2.tile([P, C], F32, name="h2_t")
                            nc.vector.tensor_scalar(
                                out=h2_t,
                                in0=x2_sb[:, m, :],
                                scalar1=mv2[:, 0:1],
                                scalar2=rstd2,
                                op0=ALU.subtract,
                                op1=ALU.mult,
                            )
                            nc.vector.tensor_mul(h2_t, h2_t, g2b)
                            nc.vector.tensor_add(h2_t, h2_t, be2b)
                            for j in range(NCT):
                                tr_ps = ph2psA.tile([P, P], F32, name="tr_ps2")
                                nc.tensor.transpose(
                                    tr_ps[:], h2_t[:, j * P : (j + 1) * P], ident[:]
                                )
                                nc.vector.tensor_copy(
                                    h2T[:, j, m * P : (m + 1) * P], tr_ps[:]
                                )

                    # FFN1: gT[f] = relu(W1[:,f]^T h2T + b1); W1/W2 streamed
                    gT = ph2w.tile([P, NFT, TB], F32R, name="gT")
                    if "ffn" in skip:
                        for m in range(NTT):
                            y_t = ph2.tile([P, C], F32, name="y_t")
                            nc.vector.tensor_copy(y_t, x2_sb[:, m, :])
                            nc.sync.dma_start(out=out[m * P : (m + 1) * P, :], in_=y_t)
                        continue
                    with (
                        tc.tile_pool(name="ph2psB", bufs=2, space="PSUM") as ph2psB,
                        tc.tile_pool(name="ph2psY", bufs=1, space="PSUM") as ph2psY,
                    ):
                        ps_y = ph2psY.tile([P, NTT, C], F32, name="ps_y")
                        for f in range(NFT):
                            ps_g = ph2psB.tile([P, TB], F32, name="ps_g")
                            for j in range(NCT):
                                nc.tensor.matmul(
                                    ps_g[:],
                                    w1_sb[:, j, f * P : (f + 1) * P],
                                    h2T[:, j, :],
                                    start=(j == 0),
                                    stop=(j == NCT - 1),
                                )
                            nc.scalar.activation(
                                out=gT[:, f, :],
                                in_=ps_g[:],
                                func=AF.Relu,
                                bias=b1cols[:, f : f + 1],
                            )
                            for m in range(NTT):
                                nc.tensor.matmul(
                                    ps_y[:, m, :],
                                    gT[:, f, m * P : (m + 1) * P],
                                    w2_sb[:, f, :],
                                    start=(f == 0),
                                    stop=(f == NFT - 1),
                                )
                        for m in range(NTT):
                            y_t = ph2.tile([P, C], F32, name="y_t")
                            nc.vector.tensor_add(y_t, ps_y[:, m, :], x2_sb[:, m, :])
                            nc.vector.tensor_add(y_t, y_t, b2b)
                            nc.sync.dma_start(out=out[m * P : (m + 1) * P, :], in_=y_t)

    _split_excess_waits(nc)
    return nc


_NC_CACHE = None


def _get_nc():
    global _NC_CACHE
    if _NC_CACHE is None:
        _NC_CACHE = _build_nc()
    return _NC_CACHE


def _make_in_maps(inputs):
    f = lambda a: np.ascontiguousarray(np.asarray(a, dtype=np.float32))
    x = f(inputs["x"]).reshape(NT, C)
    Wq, Wk, Wv = f(inputs["Wq"]), f(inputs["Wk"]), f(inputs["Wv"])
    bq, bk, bv = f(inputs["bq"]), f(inputs["bk"]), f(inputs["bv"])
    shared = {
        "wo": f(inputs["Wo"]),
        "bo": f(inputs["bo"]),
        "w1": f(inputs["W1"]),
        "b1r": f(inputs["b1"]).reshape(DF // P, P),
        "w2": f(inputs["W2"]),
        "b2": f(inputs["b2"]),
        "g1": f(inputs["g1"]),
        "be1": f(inputs["be1"]),
        "g2": f(inputs["g2"]),
        "be2": f(inputs["be2"]),
    }
    in_maps = []
    for c in range(NCORES):
        hs = slice(2 * c, 2 * c + 2)
        in_maps.append(
            {
                "xblk": x[c * TB : (c + 1) * TB],
                "wq": np.ascontiguousarray(
                    np.concatenate([Wq[2 * c], Wq[2 * c + 1]], axis=1)
                ),
                "wk": np.ascontiguousarray(
                    np.concatenate([Wk[2 * c], Wk[2 * c + 1]], axis=1)
                ),
                "wv": np.ascontiguousarray(
                    np.concatenate([Wv[2 * c], Wv[2 * c + 1]], axis=1)
                ),
                "bq": np.ascontiguousarray(bq[hs].reshape(-1)),
                "bk": np.ascontiguousarray(bk[hs].reshape(-1)),
                "bv": np.ascontiguousarray(bv[hs].reshape(-1)),
                **shared,
            }
        )
    return in_maps


def kernel(**inputs) -> np.ndarray:
    nc = _get_nc()
    in_maps = _make_in_maps(inputs)
    res = run_bass_kernel_spmd(nc, in_maps, list(range(NCORES)))
    out = np.concatenate([res.results[c]["out"] for c in range(NCORES)], axis=0)
    return out.reshape(B, T, C).astype(np.float32)



# revision 23
# speedup vs baseline: 2.9613x; 2.9613x over previous
"""Trainium2 Bass kernel for a pre-LN transformer block (B=2, T=2048, C=512,
H=16 heads, HS=32, DF=2048), distributed over 8 NeuronCores.

Sharding strategy (v2):
  Cores are split into 2 groups of 4 by batch (cores 0-3 -> batch 0,
  cores 4-7 -> batch 1). Each core:
   - Phase 0: replicates LN1 over ALL 2048 tokens of its batch (no
     collective needed; LN gain/bias are folded into the QKV weights
     host-side) and PE-transposes to hT [C, 2048] bf16.
   - Phase 1: computes q^T,k^T [128, 2048] and v for its 4 heads.
   - Phase 1b: causal attention in transposed-score space; the per-head
     softmax denominator Z comes from a ones-column in the [v|1]
     stationary; 1/Z is folded in at the source via a small PE broadcast
     matmul, so the AllToAll payload is normalized o in fp8 (256KB total).
   - AllToAll (8-core mesh): head-sharded -> token-sharded, where each
     core owns token slab [256c, 256c+256) of BOTH batches so the
     collective is fully dense.
   - Phase 2: Wo + residual + LN2 + FFN for its 512 tokens.
  All LN gains/biases and bo are folded host-side (diag(g)@W, be@W + b).
"""
import numpy as np

import bass_rust
import concourse.bass as bass
import concourse.mybir as mybir
import concourse.tile as tile
from concourse.bass_utils import run_bass_kernel_spmd

B, T, C, H, HS = 2, 2048, 512, 16, 32
DF = 4 * C
EPS = 1e-3
NCORES = 8
GROUP = 4           # cores per batch group
HPC = H // GROUP    # 4 heads per core
D2 = HPC * HS       # 128 = packed head dim per core
TB = 512            # token chunk for QKV/attention loops
QT = 256            # token slab per core for phase 2 (per batch)
P = 128
NCT = C // P        # 4 c-tiles
NFT = DF // P       # 16 f-tiles
NTT = T // P        # 16 token tiles per batch
NST = T // P        # 16 s-tiles per batch
F32 = mybir.dt.float32
F32R = mybir.dt.float32r
BF16 = mybir.dt.bfloat16
FP8 = mybir.dt.float8e4
AF = mybir.ActivationFunctionType
ALU = mybir.AluOpType

_ev_counter = [0]


def _split_excess_waits(nc, max_waits=1):
    """This walrus build rejects >1 semaphore wait per real instruction; Tile's
    kernel-tail drain (and occasionally other aggregation points) can exceed
    that. Hoist extra waits onto EventSemaphore instructions inserted
    immediately before, on the same engine."""
    n_split = 0
    for bb in nc.main_func.blocks:
        il = bb.instructions
        i = 0
        while i < len(il):
            inst = il[i]
            si = inst.sync_info
            if si is None:
                i += 1
                continue
            waits = list(si.on_wait)
            if len(waits) <= max_waits:
                i += 1
                continue
            keep, extra = waits[:max_waits], waits[max_waits:]
            evs = []
            for w in extra:
                _ev_counter[0] += 1
                ev = mybir.InstEventSemaphore(
                    name=f"EV-WSPLIT-{_ev_counter[0]}",
                    engine=inst.engine,
                    sync_info=bass_rust.SyncInfo(on_wait=[w], on_update=[]),
                )
                nc.register_instruction(ev)
                evs.append(ev)
            inst.sync_info = bass_rust.SyncInfo(
                on_wait=keep, on_update=list(si.on_update)
            )
            for k, ev in enumerate(evs):
                il.insert(i + k, ev)
            i += len(evs) + 1
            n_split += 1
    return n_split


def _build_nc(repeat=1, skip=()):
    nc = bass.Bass(num_devices=NCORES)

    # ---- per-core external inputs ----
    xfull = nc.declare_dram_parameter("xfull", [T, C], F32, isOutput=False)
    xres = nc.declare_dram_parameter("xres", [2 * QT, C], F32, isOutput=False)
    wq = nc.declare_dram_parameter("wq", [C, D2], F32, isOutput=False)
    wk = nc.declare_dram_parameter("wk", [C, D2], F32, isOutput=False)
    wv = nc.declare_dram_parameter("wv", [C, D2], F32, isOutput=False)
    bq = nc.declare_dram_parameter("bq", [D2], F32, isOutput=False)
    bk = nc.declare_dram_parameter("bk", [D2], F32, isOutput=False)
    bv = nc.declare_dram_parameter("bv", [D2], F32, isOutput=False)
    wo = nc.declare_dram_parameter("wo", [C, C], F32, isOutput=False)
    w1 = nc.declare_dram_parameter("w1", [C, DF], F32, isOutput=False)
    b1r = nc.declare_dram_parameter("b1r", [DF // P, P], F32, isOutput=False)
    w2 = nc.declare_dram_parameter("w2", [DF, C], F32, isOutput=False)
    b2 = nc.declare_dram_parameter("b2", [C], F32, isOutput=False)
    out = nc.declare_dram_parameter("out", [2 * QT, C], F32, isOutput=True)

    ident_dram = nc.inline_tensor(np.eye(P, dtype=np.float32), name="ident_c")
    # E4[r, p] = 1 iff p // 32 == r  (broadcast 1/Z row r to its 32 partitions)
    e4 = np.zeros((HPC, P), dtype=np.float32)
    for r in range(HPC):
        e4[r, 32 * r : 32 * r + 32] = 1.0
    e4_dram = nc.inline_tensor(e4, name="e4_c")

    with tile.TileContext(nc) as tc:
        import contextlib

        with contextlib.ExitStack() as ctx:
            const = ctx.enter_context(tc.tile_pool(name="const", bufs=1))
            persist = ctx.enter_context(tc.tile_pool(name="persist", bufs=1))
            dram = ctx.enter_context(tc.tile_pool(name="dram", bufs=1, space="DRAM"))

            # ---- constants ----
            identb = const.tile([P, P], BF16, name="identb")
            ident_st = const.tile([P, P], F32, name="ident_st")
            nc.sync.dma_start(out=ident_st, in_=ident_dram[:, :])
            nc.vector.tensor_copy(identb, ident_st)
            eps_t = const.tile([P, 1], F32, name="eps_t")
            nc.vector.memset(eps_t, EPS)
            e4_sb = const.tile([HPC, P], F32, name="e4_sb")
            nc.sync.dma_start(out=e4_sb, in_=e4_dram[:, :])
            ones_row = const.tile([1, P], BF16, name="ones_row")
            nc.vector.memset(ones_row, 1.0)
            b2_sb = const.tile([1, C], BF16, name="b2_sb")
            b2_st = const.tile([1, C], F32, name="b2_st")
            nc.sync.dma_start(out=b2_st, in_=b2[:].unsqueeze(0))
            nc.vector.tensor_copy(b2_sb, b2_st)
            bq_sb = const.tile([D2, 1], F32, name="bq_sb")
            nc.sync.dma_start(out=bq_sb, in_=bq[:].unsqueeze(1))
            bk_sb = const.tile([D2, 1], F32, name="bk_sb")
            nc.sync.dma_start(out=bk_sb, in_=bk[:].unsqueeze(1))
            bvb = const.tile([P, D2], F32, name="bvb")
            nc.sync.dma_start(out=bvb, in_=bv[:].partition_broadcast(P))
            b1cols = const.tile([P, NFT], F32, name="b1cols")
            for f in range(NFT):
                nc.sync.dma_start(out=b1cols[:, f : f + 1], in_=b1r[f, :].unsqueeze(1))

            # ---- weights resident in SBUF ----
            wq_sb = const.tile([P, NCT, D2], BF16, name="wq_sb")
            wk_sb = const.tile([P, NCT, D2], BF16, name="wk_sb")
            wv_sb = const.tile([P, NCT, D2], BF16, name="wv_sb")
            wqkv_st = const.tile([P, 3 * D2], F32, name="wqkv_st")
            for j in range(NCT):
                nc.sync.dma_start(out=wqkv_st[:, 0:D2], in_=wq[j * P : (j + 1) * P, :])
                nc.sync.dma_start(
                    out=wqkv_st[:, D2 : 2 * D2], in_=wk[j * P : (j + 1) * P, :]
                )
                nc.sync.dma_start(
                    out=wqkv_st[:, 2 * D2 : 3 * D2], in_=wv[j * P : (j + 1) * P, :]
                )
                nc.vector.tensor_copy(wq_sb[:, j, :], wqkv_st[:, 0:D2])
                nc.vector.tensor_copy(wk_sb[:, j, :], wqkv_st[:, D2 : 2 * D2])
                nc.vector.tensor_copy(wv_sb[:, j, :], wqkv_st[:, 2 * D2 : 3 * D2])
            wo_sb = const.tile([P, NCT, C], BF16, name="wo_sb")
            wo_st = const.tile([P, C], F32, name="wo_st")
            for j in range(NCT):
                nc.sync.dma_start(out=wo_st, in_=wo[j * P : (j + 1) * P, :])
                nc.vector.tensor_copy(wo_sb[:, j, :], wo_st)
            w1_sb = const.tile([P, NCT, DF], BF16, name="w1_sb")
            w1_st = const.tile([P, DF], F32, name="w1_st")
            for j in range(NCT):
                nc.sync.dma_start(out=w1_st, in_=w1[j * P : (j + 1) * P, :])
                nc.vector.tensor_copy(w1_sb[:, j, :], w1_st)
            w2_sb = const.tile([P, NFT, C], BF16, name="w2_sb")
            w2_st = const.tile([P, C], F32, name="w2_st")
            for f in range(NFT):
                nc.sync.dma_start(out=w2_st, in_=w2[f * P : (f + 1) * P, :])
                nc.vector.tensor_copy(w2_sb[:, f, :], w2_st)

            for _rep in range(repeat):
                # per-repeat DRAM comm buffers
                a2a_in = dram.tile([NCORES, P, QT], FP8, name="a2a_in")
                a2a_out = dram.tile([NCORES, P, QT], FP8, name="a2a_out")

                # ======== Phase 0: replicated LN1 over my batch + transpose ====
                x_sb = persist.tile([P, 2 * QT // P, C], F32, name="x_sb")  # residual
                nc.sync.dma_start(out=x_sb[:, 0, :], in_=xres[0:P, :])
                nc.sync.dma_start(out=x_sb[:, 1, :], in_=xres[P : 2 * P, :])
                nc.sync.dma_start(out=x_sb[:, 2, :], in_=xres[2 * P : 3 * P, :])
                nc.sync.dma_start(out=x_sb[:, 3, :], in_=xres[3 * P : 4 * P, :])

                p1ctx = contextlib.ExitStack()
                p1big = p1ctx.enter_context(tc.tile_pool(name="p1big", bufs=1))
                hT = p1big.tile([P, NCT, T], BF16, name="hT")
                with (
                    tc.tile_pool(name="ph0", bufs=3) as ph0,
                    tc.tile_pool(name="ph0ps", bufs=4, space="PSUM") as ph0ps,
                ):
                    for i in range(NTT):
                        x_t = ph0.tile([P, C], F32, name="x_t0")
                        nc.sync.dma_start(out=x_t, in_=xfull[i * P : (i + 1) * P, :])
                        stats = ph0.tile([P, 6], F32, name="stats0")
                        nc.vector.bn_stats(out=stats, in_=x_t)
                        mv = ph0.tile([P, 2], F32, name="mv0")
                        nc.vector.bn_aggr(out=mv, in_=stats)
                        rstd = ph0.tile([P, 1], F32, name="rstd0")
                        nc.scalar.activation(
                            out=rstd, in_=mv[:, 1:2], func=AF.Sqrt, bias=eps_t
                        )
                        nc.vector.reciprocal(out=rstd, in_=rstd)
                        nmr = ph0.tile([P, 1], F32, name="nmr0")
                        nc.vector.tensor_scalar(
                            out=nmr,
                            in0=mv[:, 0:1],
                            scalar1=rstd,
                            scalar2=-1.0,
                            op0=ALU.mult,
                            op1=ALU.mult,
                        )
                        h_t = ph0.tile([P, C], BF16, name="h_t0")
                        nc.gpsimd.tensor_scalar(
                            out=h_t,
                            in0=x_t,
                            scalar1=rstd,
                            scalar2=nmr,
                            op0=ALU.mult,
                            op1=ALU.add,
                        )
                        for j in range(NCT):
                            tr_ps = ph0ps.tile([P, P], BF16, name="tr_ps0")
                            nc.tensor.transpose(
                                tr_ps[:], h_t[:, j * P : (j + 1) * P], identb[:]
                            )
                            nc.vector.tensor_copy(
                                hT[:, j, i * P : (i + 1) * P], tr_ps[:]
                            )

                # ======== Phase 1: QKV for my 4 heads over my batch ============
                # two heads per tile: PE operand base partition must be 0/32/64
                qTt = [
                    p1big.tile([2 * HS, T], BF16, name=f"qT{a}") for a in range(2)
                ]
                kTt = [
                    p1big.tile([2 * HS, T], BF16, name=f"kT{a}") for a in range(2)
                ]
                # v layout: [128 s-part, s-tile 16, head 4, 33]; col 32 = ones
                v_sb = p1big.tile([P, NST, HPC, HS + 1], BF16, name="v_sb")
                nc.vector.memset(v_sb[:, :, :, HS : HS + 1], 1.0)
                oz = p1big.tile([P, T], BF16, name="oz")
                zrow = p1big.tile([1, HPC * T], F32, name="zrow")

                with (
                    tc.tile_pool(name="ph1", bufs=2) as ph1,
                    tc.tile_pool(name="ph1ps", bufs=2, space="PSUM") as ph1ps,
                ):
                    for n in range(T // TB):  # 4 chunks of 512 tokens
                        ps_q = ph1ps.tile([D2, TB], F32, name="ps_q")
                        for j in range(NCT):
                            nc.tensor.matmul(
                                ps_q[:],
                                wq_sb[:, j, :],
                                hT[:, j, n * TB : (n + 1) * TB],
                                start=(j == 0),
                                stop=(j == NCT - 1),
                            )
                        for a in range(2):
                            nc.vector.tensor_scalar(
                                out=qTt[a][:, n * TB : (n + 1) * TB],
                                in0=ps_q[64 * a : 64 * a + 64],
                                scalar1=bq_sb[64 * a : 64 * a + 64],
                                scalar2=None,
                                op0=ALU.add,
                            )
                        ps_k = ph1ps.tile([D2, TB], F32, name="ps_k")
                        for j in range(NCT):
                            nc.tensor.matmul(
                                ps_k[:],
                                wk_sb[:, j, :],
                                hT[:, j, n * TB : (n + 1) * TB],
                                start=(j == 0),
                                stop=(j == NCT - 1),
                            )
                        for a in range(2):
                            nc.vector.tensor_scalar(
                                out=kTt[a][:, n * TB : (n + 1) * TB],
                                in0=ps_k[64 * a : 64 * a + 64],
                                scalar1=bk_sb[64 * a : 64 * a + 64],
                                scalar2=None,
                                op0=ALU.add,
                            )
                        for m in range(TB // P):
                            g = n * (TB // P) + m  # global s-tile index
                            ps_v = ph1ps.tile([P, D2], F32, name="ps_v")
                            for j in range(NCT):
                                nc.tensor.matmul(
                                    ps_v[:],
                                    hT[:, j, g * P : (g + 1) * P],
                                    wv_sb[:, j, :],
                                    start=(j == 0),
                                    stop=(j == NCT - 1),
                                )
                            nc.vector.tensor_tensor(
                                out=v_sb[:, g, :, 0:HS],
                                in0=ps_v[:],
                                in1=bvb,
                                op=ALU.add,
                            )

                # ======== Phase 1b: causal attention, transposed-score space ===
                if "attn" in skip:
                    nc.gpsimd.memset(oz[:, 0:1024], 1.0)
                    nc.gpsimd.memset(oz[:, 1024:2048], 1.0)
                    nc.gpsimd.memset(zrow[:, :], 1.0)
                with (
                    tc.tile_pool(name="att", bufs=5) as att,
                    tc.tile_pool(name="attps", bufs=3, space="PSUM") as attps,
                    tc.tile_pool(name="attpso", bufs=2, space="PSUM") as attpso,
                ):
                    for hh in range(HPC if "attn" not in skip else 0):
                        qT = qTt[hh // 2]
                        kT = kTt[hh // 2]
                        qrows = slice((hh % 2) * HS, (hh % 2 + 1) * HS)
                        for ci in range(T // TB):  # 4 t-chunks of 512
                            t0 = ci * TB
                            ns = 4 * ci + 4  # s-tiles for this chunk
                            ps_o = attpso.tile([HS + 1, TB], F32, name="ps_o")
                            for jp in range(0, ns, 2):
                                ps_s = attps.tile([P, 2 * TB], F32, name="ps_s")
                                for u in range(2):
                                    j = jp + u
                                    nc.tensor.matmul(
                                        ps_s[:, u * TB : (u + 1) * TB],
                                        kT[qrows, j * P : (j + 1) * P],
                                        qT[qrows, t0 : t0 + TB],
                                        start=True,
                                        stop=True,
                                    )
                                e_t = att.tile([P, 2 * TB], BF16, name="e_t")
                                nc.scalar.activation(out=e_t, in_=ps_s[:], func=AF.Exp)
                                for u in range(2):
                                    j = jp + u
                                    off = j - 4 * ci
                                    if off >= 0:
                                        # mask the [128,128] diagonal block
                                        nc.gpsimd.affine_select(
                                            out=e_t[:, u * TB + off * P : u * TB + (off + 1) * P],
                                            in_=e_t[:, u * TB + off * P : u * TB + (off + 1) * P],
                                            compare_op=ALU.is_ge,
                                            fill=0.0,
                                            base=0,
                                            pattern=[[1, P]],
                                            channel_multiplier=-1,
                                        )
                                for u in range(2):
                                    j = jp + u
                                    off = max(j - 4 * ci, 0)
                                    nc.tensor.matmul(
                                        ps_o[:, off * P : TB],
                                        v_sb[:, j, hh, :],
                                        e_t[:, u * TB + off * P : (u + 1) * TB],
                                        start=(jp == 0 and u == 0),
                                        stop=(jp + u == ns - 1),
                                    )
                            nc.vector.tensor_copy(
                                oz[hh * HS : (hh + 1) * HS, t0 : t0 + TB],
                                ps_o[0:HS, :],
                            )
                            nc.vector.tensor_copy(
                                zrow[0:1, (4 * hh + ci) * TB : (4 * hh + ci + 1) * TB],
                                ps_o[HS : HS + 1, :],
                            )

                # fold 1/Z into o at the source; convert to fp8 for the A2A
                oz8 = p1big.tile([P, T], FP8, name="oz8")
                with tc.tile_pool(name="rmt", bufs=1) as rmt:
                    zd = dram.tile([HPC * T], F32, name="zd")
                    nc.sync.dma_start(out=zd[:].unsqueeze(0), in_=zrow[0:1, :])
                    zmat = rmt.tile([P, T], F32, name="zmat")
                    for hh in range(HPC):
                        for ci in range(T // TB):
                            nc.sync.dma_start(
                                out=zmat[
                                    32 * hh : 32 * hh + 32, ci * TB : (ci + 1) * TB
                                ],
                                in_=zd[
                                    (4 * hh + ci) * TB : (4 * hh + ci + 1) * TB
                                ].partition_broadcast(32),
                            )
                    rmatb = rmt.tile([P, T], BF16, name="rmatb")
                    with nc.allow_low_precision(reason="1/Z in bf16 is plenty"):
                        nc.vector.reciprocal(out=rmatb, in_=zmat)
                    nc.vector.tensor_tensor(
                        out=oz8, in0=oz, in1=rmatb, op=ALU.mult
                    )

                # pack + AllToAll (dst k owns tokens [256k, 256k+256) of my batch)
                for k in range(NCORES):
                    nc.sync.dma_start(
                        out=a2a_in[k, :, :], in_=oz8[:, k * QT : (k + 1) * QT]
                    )
                p1ctx.close()
                if "a2a" in skip:
                    nc.sync.dma_start(out=a2a_out[:, :, :], in_=a2a_in[:, :, :])
                else:
                    nc.gpsimd.collective_compute(
                        "AllToAll",
                        ALU.bypass,
                        replica_groups=[list(range(NCORES))],
                        ins=[a2a_in[:, :, :]],
                        outs=[a2a_out[:, :, :]],
                    )

                # ======== Phase 2: Wo + residual + LN2 + FFN ===================
                with (
                    tc.tile_pool(name="ph2", bufs=2) as ph2,
                    tc.tile_pool(name="ph2w", bufs=1) as ph2w,
                ):
                    x2_sb = ph2w.tile([P, 2 * QT // P, C], F32, name="x2_sb")
                    h2T = ph2w.tile([P, NCT, 2 * QT], BF16, name="h2T")
                    with tc.tile_pool(name="ph2psA", bufs=4, space="PSUM") as ph2psA:
                        for s2 in range(2):  # batch slab
                            # unpack: oT[32*jsrc+hs, hh, col] =
                            #   a2a_out[4*s2+jsrc, 32*hh+hs, col]
                            oT8 = ph2.tile([P, HPC, QT], FP8, name="oT8")
                            for jsrc in range(GROUP):
                                nc.sync.dma_start(
                                    out=oT8[32 * jsrc : 32 * jsrc + 32, :, :],
                                    in_=a2a_out[s2 * GROUP + jsrc, :, :].rearrange(
                                        "(h s) c -> s h c", h=HPC, s=HS
                                    ),
                                )
                            oTb = ph2.tile([P, HPC, QT], BF16, name="oTb")
                            nc.vector.tensor_copy(oTb, oT8)
                            for m in range(QT // P):
                                mi = 2 * s2 + m  # tile index within my 512 tokens
                                ps_a = ph2psA.tile([P, C], F32, name="ps_a")
                                for blk in range(HPC):
                                    nc.tensor.matmul(
                                        ps_a[:],
                                        oTb[:, blk, m * P : (m + 1) * P],
                                        wo_sb[:, blk, :],
                                        start=(blk == 0),
                                        stop=(blk == HPC - 1),
                                    )
                                nc.vector.tensor_tensor(
                                    out=x2_sb[:, mi, :],
                                    in0=ps_a[:],
                                    in1=x_sb[:, mi, :],
                                    op=ALU.add,
                                )
                                stats2 = ph2.tile([P, 6], F32, name="stats2")
                                nc.vector.bn_stats(out=stats2, in_=x2_sb[:, mi, :])
                                mv2 = ph2.tile([P, 2], F32, name="mv2")
                                nc.vector.bn_aggr(out=mv2, in_=stats2)
                                rstd2 = ph2.tile([P, 1], F32, name="rstd2")
                                nc.scalar.activation(
                                    out=rstd2, in_=mv2[:, 1:2], func=AF.Sqrt, bias=eps_t
                                )
                                nc.vector.reciprocal(out=rstd2, in_=rstd2)
                                nmr2 = ph2.tile([P, 1], F32, name="nmr2")
                                nc.vector.tensor_scalar(
                                    out=nmr2,
                                    in0=mv2[:, 0:1],
                                    scalar1=rstd2,
                                    scalar2=-1.0,
                                    op0=ALU.mult,
                                    op1=ALU.mult,
                                )
                                h2_t = ph2.tile([P, C], BF16, name="h2_t")
                                nc.gpsimd.tensor_scalar(
                                    out=h2_t,
                                    in0=x2_sb[:, mi, :],
                                    scalar1=rstd2,
                                    scalar2=nmr2,
                                    op0=ALU.mult,
                                    op1=ALU.add,
                                )
                                for j in range(NCT):
                                    tr_ps = ph2psA.tile([P, P], BF16, name="tr_ps2")
                                    nc.tensor.transpose(
                                        tr_ps[:], h2_t[:, j * P : (j + 1) * P], identb[:]
                                    )
                                    nc.vector.tensor_copy(
                                        h2T[:, j, mi * P : (mi + 1) * P], tr_ps[:]
                                    )

                    # FFN over my 512 tokens
                    gT = ph2w.tile([P, NFT, 2 * QT], BF16, name="gT")
                    if "ffn" in skip:
                        for m in range(2 * QT // P):
                            y_t = ph2.tile([P, C], F32, name="y_t")
                            nc.vector.tensor_copy(y_t, x2_sb[:, m, :])
                            nc.sync.dma_start(out=out[m * P : (m + 1) * P, :], in_=y_t)
                        continue
                    with (
                        tc.tile_pool(name="ph2psB", bufs=2, space="PSUM") as ph2psB,
                        tc.tile_pool(name="ph2psY", bufs=1, space="PSUM") as ph2psY,
                    ):
                        ps_y = ph2psY.tile([P, 2 * QT // P, C], F32, name="ps_y")
                        for f in range(NFT):
                            ps_g = ph2psB.tile([P, 2 * QT], F32, name="ps_g")
                            for j in range(NCT):
                                nc.tensor.matmul(
                                    ps_g[:],
                                    w1_sb[:, j, f * P : (f + 1) * P],
                                    h2T[:, j, :],
                                    start=(j == 0),
                                    stop=(j == NCT - 1),
                                )
                            nc.scalar.activation(
                                out=gT[:, f, :],
                                in_=ps_g[:],
                                func=AF.Relu,
                                bias=b1cols[:, f : f + 1],
                            )
                            for m in range(2 * QT // P):
                                nc.tensor.matmul(
                                    ps_y[:, m, :],
                                    gT[:, f, m * P : (m + 1) * P],
                                    w2_sb[:, f, :],
                                    start=(f == 0),
                                    stop=False,
                                )
                        for m in range(2 * QT // P):
                            # rank-1 b2 add closes the accumulation group
                            nc.tensor.matmul(
                                ps_y[:, m, :],
                                ones_row[0:1, 0:P],
                                b2_sb[0:1, :],
                                start=False,
                                stop=True,
                            )
                            y_t = ph2.tile([P, C], F32, name="y_t")
                            nc.vector.tensor_tensor(
                                out=y_t, in0=ps_y[:, m, :], in1=x2_sb[:, m, :], op=ALU.add
                            )
                            nc.sync.dma_start(out=out[m * P : (m + 1) * P, :], in_=y_t)

    _split_excess_waits(nc)
    return nc


_NC_CACHE = None


def _get_nc():
    global _NC_CACHE
    if _NC_CACHE is None:
        _NC_CACHE = _build_nc()
    return _NC_CACHE


def _make_in_maps(inputs):
    f = lambda a: np.ascontiguousarray(np.asarray(a, dtype=np.float32))
    x = f(inputs["x"])  # [B, T, C]
    Wq, Wk, Wv = f(inputs["Wq"]), f(inputs["Wk"]), f(inputs["Wv"])
    bq, bk, bv = f(inputs["bq"]), f(inputs["bk"]), f(inputs["bv"])
    Wo, bo = f(inputs["Wo"]), f(inputs["bo"])
    W1, b1 = f(inputs["W1"]), f(inputs["b1"])
    W2, b2 = f(inputs["W2"]), f(inputs["b2"])
    g1, be1 = f(inputs["g1"]), f(inputs["be1"])
    g2, be2 = f(inputs["g2"]), f(inputs["be2"])

    # LN1 fold: h = z*g1 + be1  =>  h@W = z@(g1 d W) + be1@W
    Wq_f = g1[:, None] * Wq  # [H, C, HS] broadcast over H? shape [H,C,HS]
    Wk_f = g1[:, None] * Wk
    Wv_f = g1[:, None] * Wv
    # per-head folded biases
    bq_f = np.einsum("c,hcd->hd", be1, Wq) + bq
    bk_f = np.einsum("c,hcd->hd", be1, Wk) + bk
    bv_f = np.einsum("c,hcd->hd", be1, Wv) + bv
    # LN2 fold into FFN1
    W1_f = g2[:, None] * W1
    b1_f = be2 @ W1 + b1
    # wo row permutation: c' = 128*hh + 32*jsrc + hs <- c = 32*(4*jsrc+hh)+hs
    perm = np.empty(C, dtype=np.int64)
    for hh in range(HPC):
        for jsrc in range(GROUP):
            for hs in range(HS):
                perm[128 * hh + 32 * jsrc + hs] = 32 * (4 * jsrc + hh) + hs
    Wo_p = np.ascontiguousarray(Wo[perm])

    shared = {
        "wo": Wo_p,
        "w1": np.ascontiguousarray(W1_f),
        "b1r": np.ascontiguousarray(b1_f).reshape(DF // P, P),
        "w2": W2,
        "b2": b2,
    }
    in_maps = []
    for c in range(NCORES):
        g = c // GROUP
        h0 = HPC * (c % GROUP)
        hsl = slice(h0, h0 + HPC)
        xres = np.concatenate(
            [x[0, QT * c : QT * (c + 1)], x[1, QT * c : QT * (c + 1)]], axis=0
        ) + bo[None, :]
        in_maps.append(
            {
                "xfull": x[g],
                "xres": np.ascontiguousarray(xres),
                "wq": np.ascontiguousarray(
                    Wq_f[hsl].transpose(1, 0, 2).reshape(C, D2)
                ),
                "wk": np.ascontiguousarray(
                    Wk_f[hsl].transpose(1, 0, 2).reshape(C, D2)
                ),
                "wv": np.ascontiguousarray(
                    Wv_f[hsl].transpose(1, 0, 2).reshape(C, D2)
                ),
                "bq": np.ascontiguousarray(bq_f[hsl].reshape(-1)),
                "bk": np.ascontiguousarray(bk_f[hsl].reshape(-1)),
                "bv": np.ascontiguousarray(bv_f[hsl].reshape(-1)),
                **shared,
            }
        )
    return in_maps


def kernel(**inputs) -> np.ndarray:
    nc = _get_nc()
    in_maps = _make_in_maps(inputs)
    res = run_bass_kernel_spmd(nc, in_maps, list(range(NCORES)))
    out = np.empty((B, T, C), dtype=np.float32)
    for c in range(NCORES):
        r = res.results[c]["out"]
        out[0, QT * c : QT * (c + 1)] = r[0:QT]
        out[1, QT * c : QT * (c + 1)] = r[QT : 2 * QT]
    return out
